# revision 47
# baseline (speedup 1.0000x reference)
"""Trainium2 Bass kernel for nn_GCN_23029614641773.

The reference GCN operates on B independent 27-node graphs where every node of
graph i starts with the same feature vector x[i], and only node 0 of each graph
feeds the classifier head. Exploiting linearity of the edge aggregation, the
whole network collapses exactly (up to fp rounding order) to a per-sample MLP:

    y = x @ W0                                  # [B, 1024]
    s = lrelu(y + b0) + 2*lrelu(3y + b0) + lrelu(5y + b0)
      # node 1's in-neighbours {0,2,4,6} have in-degrees {1,3,3,5}.
      # With b0 == 0 (spec fill): s == 12*lrelu(y) exactly.
    t = s @ W1;  h = lrelu(t + b1)              # [B, 512]
    v = h @ W2;  g = lrelu(v + b2)              # [B, 256]
    out = g @ Wc + bc                           # [B, 1]

Sharding: pure data parallelism, batch split across 8 NeuronCores; each core
holds the full weight set.

Perf design (memory-regime):
  * All operands cast to fp16 on the host (free) -> halves HBM traffic to
    ~3.8 MB/core; fp16 matmuls stream 1 col/cycle on the PE like f32r.
  * x is transposed and weights are pre-tiled on the host into exactly the
    SBUF layout the PE wants (no on-chip transposes, no eye matrix).
  * One strictly-serial sync-ring DMA stream in consumption order
    (W0[m0], x, W0[m1..m7] fine-grained, W1, W2, Wc): back-to-back
    completion receipts pipeline and layer-1 starts ~4.5us after the
    first descriptor hits the wire.
  * The stream's last two slots eat a ~4us completion-semaphore lag, so
    they carry W2 (gates only the short c3->g->cls chain) and Wc (gates
    nothing extra).
  * A burst of dummy matmuls warms the PE HAM clock gate during the
    initial DMA window so every real matmul runs at 2.4 GHz; the 106
    real matmuls then stream back-to-back at the PE roofline.
  * Tail: g0's Prelu on the scalar engine in parallel with g1's lrelu on
    the DVE, classifier accumulates as each becomes ready.
"""

from contextlib import ExitStack

import numpy as np

import concourse.bacc as bacc
import concourse.mybir as mybir
import concourse.tile as tile
from concourse.bass_utils import run_bass_kernel_spmd

F32 = mybir.dt.float32
F16 = mybir.dt.float16
P = 128
N_CORES = 8
B_FULL = 2048
B = B_FULL // N_CORES  # 256 rows per core
D0, D1, D2, D3 = 1024, 1024, 512, 256
K0, M0 = D0 // P, D1 // P  # 8, 8
K1, M1 = D1 // P, D2 // P  # 8, 4
K2, M2 = D2 // P, D3 // P  # 4, 2
KC = D3 // P  # 2

NEG_SLOPE = 0.2
N_WARMUP = 40  # dummy N=128 matmuls to warm the PE clock gate (~4.3us cold)
WU_N = 128


def _build(zero_bias: bool):
    nc = bacc.Bacc(
        "TRN2", target_bir_lowering=False, debug=False,
        enable_asserts=False, num_devices=1,
    )

    # Host-packed layouts (see kernel() below):
    #   x : [128, K0*B]       col c*B+b       = x[b, c*128+p]
    #   W0: [128, M0*K0*128]  col m*1024+c*128+f = W0[c*128+p, m*128+f]
    #   W1: [128, M1*K1*128]  likewise
    #   W2: [128, M2*K2*128]  likewise
    #   Wc: [128, KC]         col c           = Wc[c*128+p, 0]
    x_d = nc.dram_tensor("x", [P, K0 * B], F16, kind="ExternalInput").ap()
    w0_d = nc.dram_tensor("W0", [P, M0 * K0 * P], F16, kind="ExternalInput").ap()
    w1_d = nc.dram_tensor("W1", [P, M1 * K1 * P], F16, kind="ExternalInput").ap()
    w2_d = nc.dram_tensor("W2", [P, M2 * K2 * P], F16, kind="ExternalInput").ap()
    wc_d = nc.dram_tensor("Wc", [P, KC], F16, kind="ExternalInput").ap()
    if not zero_bias:
        b0_d = nc.dram_tensor("b0", [D1], F32, kind="ExternalInput").ap()
        b1_d = nc.dram_tensor("b1", [D2], F32, kind="ExternalInput").ap()
        b2_d = nc.dram_tensor("b2", [D3], F32, kind="ExternalInput").ap()
        bc_d = nc.dram_tensor("bc", [1], F32, kind="ExternalInput").ap()
    out_d = nc.dram_tensor("out", [1, B], F32, kind="ExternalOutput").ap()

    with ExitStack() as ctx:
        tc = ctx.enter_context(tile.TileContext(nc))
        const = ctx.enter_context(tc.tile_pool(name="const", bufs=1))
        xt_p = ctx.enter_context(tc.tile_pool(name="xt", bufs=1))
        w0_p = ctx.enter_context(tc.tile_pool(name="w0", bufs=M0))
        w1_p = ctx.enter_context(tc.tile_pool(name="w1", bufs=2))
        w2_p = ctx.enter_context(tc.tile_pool(name="w2", bufs=1))
        wc_p = ctx.enter_context(tc.tile_pool(name="wc", bufs=1))
        s_p = ctx.enter_context(tc.tile_pool(name="s", bufs=K1))
        h_p = ctx.enter_context(tc.tile_pool(name="h", bufs=K2))
        g_p = ctx.enter_context(tc.tile_pool(name="g", bufs=KC))
        out_p = ctx.enter_context(tc.tile_pool(name="outp", bufs=1))
        tmp_p = None
        if not zero_bias:
            tmp_p = ctx.enter_context(tc.tile_pool(name="tmp", bufs=4))
        ps_p = ctx.enter_context(tc.tile_pool(name="ps", bufs=4, space="PSUM"))
        g_psp = ctx.enter_context(tc.tile_pool(name="gps", bufs=2, space="PSUM"))
        wu_ps = ctx.enter_context(tc.tile_pool(name="wups", bufs=1, space="PSUM"))
        cls_ps = ctx.enter_context(tc.tile_pool(name="cls", bufs=1, space="PSUM"))

        # leaky-relu slope as a per-partition alpha vector for ACT Prelu
        alt = const.tile([P, 1], F32, tag="alt")
        nc.vector.memset(alt[:], NEG_SLOPE)
        # warmup operand
        wu = const.tile([P, WU_N], F16, tag="wu")
        nc.vector.memset(wu[:], 0.0)

        # ---- DMA stream: everything on the sync ring, strictly serial and
        # in consumption order. Each dma_start costs ~0.65us of issuing-
        # engine NX time (DMA_DIRECT2D) but issue of chunk k+1 hides under
        # the data streaming of chunk k; back-to-back completion receipts
        # pipeline, so serial beats splitting across two rings (which
        # halves the bandwidth of the gating first transfers).
        xt0 = xt_p.tile([P, K0 * B], F16, tag="xt", name="xt0")
        w0m, w1m, w2m = [], [], []

        def wtiles(pool, w_dram, K, groups, lst, name, m0=0):
            # groups: list of m-chunk group sizes; one dma_start per group
            for gi, g in enumerate(groups):
                t = pool.tile([P, g * K * P], F16, tag="w",
                              name=f"{name}_{gi}")
                nc.sync.dma_start(
                    t[:], w_dram[:, m0 * K * P:(m0 + g) * K * P])
                for j in range(g):
                    lst.append(t[:, j * K * P:(j + 1) * K * P])
                m0 += g

        wtiles(w0_p, w0_d, K0, [1], w0m, "w0first")
        nc.sync.dma_start(xt0[:], x_d)
        wtiles(w0_p, w0_d, K0, [1] * (M0 - 1), w0m, "w0", m0=1)
        wtiles(w1_p, w1_d, K1, [2, 2], w1m, "w1")
        # The completion semaphores of the stream's last ~2 chunks land
        # ~4us after their data (fabric-enqueue outruns the HBM drain, plus
        # the write receipt), so those slots must gate the SHORTEST chains:
        # W2 second-to-last gates only c3 -> g-ACT -> cls; Wc last gates
        # nothing that isn't already waiting on the g activations.
        wtiles(w2_p, w2_d, K2, [2], w2m, "w2")
        wc = wc_p.tile([P, KC], F16)
        nc.sync.dma_start(wc[:], wc_d)

        if not zero_bias:
            b0t = const.tile([P, M0], F32, tag="b0t")
            nc.scalar.dma_start(b0t[:], b0_d.rearrange("(c p) -> p c", p=P))
            b1t = const.tile([P, M1], F32, tag="b1t")
            nc.scalar.dma_start(b1t[:], b1_d.rearrange("(c p) -> p c", p=P))
            b2t = const.tile([P, M2], F32, tag="b2t")
            nc.scalar.dma_start(b2t[:], b2_d.rearrange("(c p) -> p c", p=P))
            bct = const.tile([1, 1], F32, tag="bct")
            nc.scalar.dma_start(bct[:], bc_d.rearrange("(a b) -> a b", a=1))
            b0t2 = const.tile([P, M0], F32, tag="b0t2")
            nc.vector.tensor_scalar_mul(b0t2[:], b0t[:], 2.0)

        # ---- PE warmup: one long accumulation group of cheap matmuls keeps
        # the PE busy through the HAM cold window while the first DMAs land.
        pw = wu_ps.tile([P, WU_N], F32)
        for i in range(N_WARMUP):
            nc.tensor.matmul(pw[:], lhsT=wu[:, 0:P], rhs=wu[:],
                             start=(i == 0), stop=(i == N_WARMUP - 1))

        PRELU = mybir.ActivationFunctionType.Prelu
        xrhs = [xt0[:, c * B:(c + 1) * B] for c in range(K0)]

        def act(o, ps, scale, bias, bias2, m):
            if zero_bias:
                nc.scalar.activation(o[:], ps[:], PRELU,
                                     scale=scale, alpha=alt[:])
            elif scale == 1.0:
                nc.scalar.activation(o[:], ps[:], PRELU,
                                     bias=bias[:, m:m + 1], alpha=alt[:])
            else:
                # s = lrelu(y+b0) + lrelu(6y+2b0) + lrelu(5y+b0)
                acc = tmp_p.tile([P, B], F32, tag="acc", name=f"acc_{m}")
                first = True
                for sc, bt in ((1.0, bias), (6.0, bias2), (5.0, bias)):
                    l = tmp_p.tile([P, B], F32, tag="l", name=f"l_{m}_{sc}")
                    nc.scalar.activation(l[:], ps[:], PRELU, scale=sc,
                                         bias=bt[:, m:m + 1], alpha=alt[:])
                    if first:
                        acc, l = l, acc
                        first = False
                    else:
                        nc.vector.tensor_add(acc[:], acc[:], l[:])
                nc.vector.tensor_copy(o[:], acc[:])

        def mm_group(ps, wm_m, rhs, cs, start, stop):
            for i, c in enumerate(cs):
                nc.tensor.matmul(
                    ps[:], lhsT=wm_m[:, c * P:(c + 1) * P], rhs=rhs[c],
                    start=(start and i == 0), stop=(stop and i == len(cs) - 1),
                )

        def layer(M, K, wm, rhs, out_pool, scale, bias, bias2, lname):
            outs = []
            for m in range(M):
                ps = ps_p.tile([P, B], F32, tag="ps", name=f"ps_{lname}_{m}")
                mm_group(ps, wm[m], rhs, range(K), True, True)
                o = out_pool.tile([P, B], F16, tag=lname, name=f"{lname}_{m}")
                act(o, ps, scale, bias, bias2, m)
                outs.append(o)
            return outs

        if zero_bias:
            b0t = b0t2 = b1t = b2t = None
        s = layer(M0, K0, w0m, xrhs, s_p, 12.0, b0t, b0t2, "s")

        # ---- layers 2+3: plain loops; L3's c-ascending order means only
        # the c=3 matmuls wait on h3, and the tail is g0-ACT (scalar) in
        # parallel with g1 (DVE), then the classifier.
        srhs = [t[:] for t in s]
        h = layer(M1, K1, w1m, srhs, h_p, 1.0, b1t, None, "h")
        hrhs = [t[:] for t in h]
        g = []
        for m in range(M2):
            ps = g_psp.tile([P, B], F32, tag="gps", name=f"ps_g_{m}")
            mm_group(ps, w2m[m], hrhs, range(K2), True, True)
            o = g_p.tile([P, B], F16, tag="g", name=f"g_{m}")
            if zero_bias and m == 1:
                # g1's lrelu on the DVE (PSUM copy + max) in parallel with
                # g0's scalar-engine Prelu — shortens the serial tail
                c = g_p.tile([P, B], F32, tag="gc", name="g1c")
                nc.vector.tensor_copy(c[:], ps[:])
                nc.vector.scalar_tensor_tensor(
                    o[:], c[:], NEG_SLOPE, c[:],
                    mybir.AluOpType.mult, mybir.AluOpType.max)
            else:
                act(o, ps, 1.0, b2t, None, m)
            g.append(o[:])

        # ---- classifier: out[1, B] = sum_c Wc[c].T @ g[c] (+ bc) ----
        po = cls_ps.tile([1, B], F32)
        for c in range(KC):
            nc.tensor.matmul(
                po[:], lhsT=wc[:, c:c + 1], rhs=g[c],
                start=(c == 0), stop=(c == KC - 1),
            )
        ob = out_p.tile([1, B], F32)
        if zero_bias:
            nc.vector.tensor_copy(ob[:], po[:])
        else:
            nc.vector.tensor_scalar_add(ob[:], po[:], bct[:, 0:1])
        # out rides the (otherwise idle) scalar HWDGE ring so it doesn't
        # queue behind the weight stream's completion descriptors
        nc.scalar.dma_start(out_d, ob[:])

    nc.compile()
    return nc


_CACHE = {}


def _get_nc(zero_bias: bool):
    if zero_bias not in _CACHE:
        _CACHE[zero_bias] = _build(zero_bias)
    return _CACHE[zero_bias]


def _pack_w(w, K, M):
    # [K*128, M*128] -> [128, M*K*128] with col m*K*128 + c*128 + f
    return np.ascontiguousarray(
        w.reshape(K, P, M, P).transpose(1, 2, 0, 3).reshape(P, M * K * P)
    ).astype(np.float16)


def _run(inputs, trace=False, **kw):
    def f32(a):
        return np.ascontiguousarray(np.asarray(a), dtype=np.float32)

    x = f32(inputs["x"])
    W0, b0 = f32(inputs["W0"]), f32(inputs["b0"])
    W1, b1 = f32(inputs["W1"]), f32(inputs["b1"])
    W2, b2 = f32(inputs["W2"]), f32(inputs["b2"])
    Wc, bc = f32(inputs["Wc"]), f32(inputs["bc"])
    zero_bias = not (b0.any() or b1.any() or b2.any() or bc.any())
    nc = _get_nc(zero_bias)

    w0p = _pack_w(W0, K0, M0)
    w1p = _pack_w(W1, K1, M1)
    w2p = _pack_w(W2, K2, M2)
    wcp = np.ascontiguousarray(Wc.reshape(KC, P).T).astype(np.float16)

    in_maps = []
    for i in range(N_CORES):
        xs = x[i * B:(i + 1) * B]  # [B, D0]
        xp = np.ascontiguousarray(
            xs.reshape(B, K0, P).transpose(2, 1, 0).reshape(P, K0 * B)
        ).astype(np.float16)
        m = {"x": xp, "W0": w0p, "W1": w1p, "W2": w2p, "Wc": wcp}
        if not zero_bias:
            m.update({"b0": b0, "b1": b1, "b2": b2, "bc": bc})
        in_maps.append(m)
    res = run_bass_kernel_spmd(nc, in_maps, list(range(N_CORES)),
                               trace=trace, **kw)
    out = np.empty((B_FULL, 1), dtype=np.float32)
    for i in range(N_CORES):
        out[i * B:(i + 1) * B, 0] = res.results[i]["out"][0]
    return out, res


def kernel(**inputs) -> np.ndarray:
    out, _ = _run(inputs)
    return out


# revision 53
# speedup vs baseline: 1.0128x; 1.0128x over previous
"""Trainium2 Bass kernel for nn_GCN_23029614641773.

The reference GCN operates on B independent 27-node graphs where every node of
graph i starts with the same feature vector x[i], and only node 0 of each graph
feeds the classifier head. Exploiting linearity of the edge aggregation, the
whole network collapses exactly (up to fp rounding order) to a per-sample MLP:

    y = x @ W0                                  # [B, 1024]
    s = lrelu(y + b0) + 2*lrelu(3y + b0) + lrelu(5y + b0)
      # node 1's in-neighbours {0,2,4,6} have in-degrees {1,3,3,5}.
      # With b0 == 0 (spec fill): s == 12*lrelu(y) exactly.
    t = s @ W1;  h = lrelu(t + b1)              # [B, 512]
    v = h @ W2;  g = lrelu(v + b2)              # [B, 256]
    out = g @ Wc + bc                           # [B, 1]

Sharding: pure data parallelism, batch split across 8 NeuronCores; each core
holds the full weight set.

Perf design (memory-regime):
  * All operands cast to fp16 on the host (free) -> halves HBM traffic to
    ~3.8 MB/core; fp16 matmuls stream 1 col/cycle on the PE like f32r.
  * x is transposed and weights are pre-tiled on the host into exactly the
    SBUF layout the PE wants (no on-chip transposes, no eye matrix).
  * One strictly-serial sync-ring DMA stream in consumption order
    (W0[m0], x, W0[m1..m7] fine-grained, W1, W2, Wc): back-to-back
    completion receipts pipeline and layer-1 starts ~4.5us after the
    first descriptor hits the wire.
  * The stream's last two slots eat a ~4us completion-semaphore lag, so
    they carry W2 (gates only the short c3->g->cls chain) and Wc (gates
    nothing extra).
  * A burst of dummy matmuls warms the PE HAM clock gate during the
    initial DMA window so every real matmul runs at 2.4 GHz; the 106
    real matmuls then stream back-to-back at the PE roofline.
  * Tail: g0's Prelu on the scalar engine in parallel with g1's lrelu on
    the DVE, classifier accumulates as each becomes ready.
"""

from contextlib import ExitStack

import numpy as np

import concourse.bacc as bacc
import concourse.mybir as mybir
import concourse.tile as tile
from concourse.bass_utils import run_bass_kernel_spmd

F32 = mybir.dt.float32
F16 = mybir.dt.float16
P = 128
N_CORES = 8
B_FULL = 2048
B = B_FULL // N_CORES  # 256 rows per core
D0, D1, D2, D3 = 1024, 1024, 512, 256
K0, M0 = D0 // P, D1 // P  # 8, 8
K1, M1 = D1 // P, D2 // P  # 8, 4
K2, M2 = D2 // P, D3 // P  # 4, 2
KC = D3 // P  # 2

NEG_SLOPE = 0.2
N_WARMUP = 40  # dummy N=128 matmuls to warm the PE clock gate (~4.3us cold)
WU_N = 128


def _build(zero_bias: bool):
    nc = bacc.Bacc(
        "TRN2", target_bir_lowering=False, debug=False,
        enable_asserts=False, num_devices=1,
    )

    # Host-packed layouts (see kernel() below):
    #   x : [128, K0*B]       col c*B+b       = x[b, c*128+p]
    #   W0: [128, M0*K0*128]  col m*1024+c*128+f = W0[c*128+p, m*128+f]
    #   W1: [128, M1*K1*128]  likewise
    #   W2: [128, M2*K2*128]  likewise
    #   Wc: [128, KC]         col c           = Wc[c*128+p, 0]
    x_d = nc.dram_tensor("x", [P, K0 * B], F16, kind="ExternalInput").ap()
    w0_d = nc.dram_tensor("W0", [P, M0 * K0 * P], F16, kind="ExternalInput").ap()
    w1_d = nc.dram_tensor("W1", [P, M1 * K1 * P], F16, kind="ExternalInput").ap()
    w2_d = nc.dram_tensor("W2", [P, M2 * K2 * P], F16, kind="ExternalInput").ap()
    wc_d = nc.dram_tensor("Wc", [P, KC], F16, kind="ExternalInput").ap()
    if not zero_bias:
        b0_d = nc.dram_tensor("b0", [D1], F32, kind="ExternalInput").ap()
        b1_d = nc.dram_tensor("b1", [D2], F32, kind="ExternalInput").ap()
        b2_d = nc.dram_tensor("b2", [D3], F32, kind="ExternalInput").ap()
        bc_d = nc.dram_tensor("bc", [1], F32, kind="ExternalInput").ap()
    out_d = nc.dram_tensor("out", [1, B], F32, kind="ExternalOutput").ap()

    with ExitStack() as ctx:
        tc = ctx.enter_context(tile.TileContext(nc))
        const = ctx.enter_context(tc.tile_pool(name="const", bufs=1))
        xt_p = ctx.enter_context(tc.tile_pool(name="xt", bufs=2))
        w0_p = ctx.enter_context(tc.tile_pool(name="w0", bufs=M0))
        w1_p = ctx.enter_context(tc.tile_pool(name="w1", bufs=2))
        w2_p = ctx.enter_context(tc.tile_pool(name="w2", bufs=1))
        wc_p = ctx.enter_context(tc.tile_pool(name="wc", bufs=1))
        s_p = ctx.enter_context(tc.tile_pool(name="s", bufs=K1))
        h_p = ctx.enter_context(tc.tile_pool(name="h", bufs=K2))
        g_p = ctx.enter_context(tc.tile_pool(name="g", bufs=KC))
        out_p = ctx.enter_context(tc.tile_pool(name="outp", bufs=1))
        tmp_p = None
        if not zero_bias:
            tmp_p = ctx.enter_context(tc.tile_pool(name="tmp", bufs=4))
        ps_p = ctx.enter_context(tc.tile_pool(name="ps", bufs=4, space="PSUM"))
        g_psp = ctx.enter_context(tc.tile_pool(name="gps", bufs=2, space="PSUM"))
        wu_ps = ctx.enter_context(tc.tile_pool(name="wups", bufs=1, space="PSUM"))
        cls_ps = ctx.enter_context(tc.tile_pool(name="cls", bufs=1, space="PSUM"))

        # leaky-relu slope as a per-partition alpha vector for ACT Prelu
        alt = const.tile([P, 1], F32, tag="alt")
        nc.vector.memset(alt[:], NEG_SLOPE)
        # warmup operand
        wu = const.tile([P, WU_N], F16, tag="wu")
        nc.vector.memset(wu[:], 0.0)

        # ---- DMA stream: everything on the sync ring, strictly serial and
        # in consumption order. Each dma_start costs ~0.65us of issuing-
        # engine NX time (DMA_DIRECT2D) but issue of chunk k+1 hides under
        # the data streaming of chunk k; back-to-back completion receipts
        # pipeline, so serial beats splitting across two rings (which
        # halves the bandwidth of the gating first transfers).
        HB = K0 * B // 2
        xt0 = xt_p.tile([P, HB], F16, tag="xt", name="xt0")
        xt1 = xt_p.tile([P, HB], F16, tag="xt", name="xt1")
        w0m, w1m, w2m = [], [], []

        def wtiles(pool, w_dram, K, groups, lst, name, m0=0):
            # groups: list of m-chunk group sizes; one dma_start per group
            for gi, g in enumerate(groups):
                t = pool.tile([P, g * K * P], F16, tag="w",
                              name=f"{name}_{gi}")
                nc.sync.dma_start(
                    t[:], w_dram[:, m0 * K * P:(m0 + g) * K * P])
                for j in range(g):
                    lst.append(t[:, j * K * P:(j + 1) * K * P])
                m0 += g

        wtiles(w0_p, w0_d, K0, [1], w0m, "w0first")
        # x in two serial chunks: the first half's completion semaphore
        # lands ~0.7us earlier, so layer-1 m0's first four matmuls start
        # at xt0's sem instead of waiting for all of x
        nc.sync.dma_start(xt0[:], x_d[:, 0:HB])
        nc.sync.dma_start(xt1[:], x_d[:, HB:2 * HB])
        wtiles(w0_p, w0_d, K0, [1] * (M0 - 1), w0m, "w0", m0=1)
        wtiles(w1_p, w1_d, K1, [2, 2], w1m, "w1")
        # The completion semaphores of the stream's last ~2 chunks land
        # ~4us after their data (fabric-enqueue outruns the HBM drain, plus
        # the write receipt), so those slots must gate the SHORTEST chains:
        # W2 second-to-last gates only c3 -> g-ACT -> cls; Wc last gates
        # nothing that isn't already waiting on the g activations.
        wtiles(w2_p, w2_d, K2, [2], w2m, "w2")
        wc = wc_p.tile([P, KC], F16)
        nc.sync.dma_start(wc[:], wc_d)

        if not zero_bias:
            b0t = const.tile([P, M0], F32, tag="b0t")
            nc.scalar.dma_start(b0t[:], b0_d.rearrange("(c p) -> p c", p=P))
            b1t = const.tile([P, M1], F32, tag="b1t")
            nc.scalar.dma_start(b1t[:], b1_d.rearrange("(c p) -> p c", p=P))
            b2t = const.tile([P, M2], F32, tag="b2t")
            nc.scalar.dma_start(b2t[:], b2_d.rearrange("(c p) -> p c", p=P))
            bct = const.tile([1, 1], F32, tag="bct")
            nc.scalar.dma_start(bct[:], bc_d.rearrange("(a b) -> a b", a=1))
            b0t2 = const.tile([P, M0], F32, tag="b0t2")
            nc.vector.tensor_scalar_mul(b0t2[:], b0t[:], 2.0)

        # ---- PE warmup: one long accumulation group of cheap matmuls keeps
        # the PE busy through the HAM cold window while the first DMAs land.
        pw = wu_ps.tile([P, WU_N], F32)
        for i in range(N_WARMUP):
            nc.tensor.matmul(pw[:], lhsT=wu[:, 0:P], rhs=wu[:],
                             start=(i == 0), stop=(i == N_WARMUP - 1))

        PRELU = mybir.ActivationFunctionType.Prelu
        xrhs = [xt0[:, c * B:(c + 1) * B] for c in range(K0 // 2)] + \
               [xt1[:, c * B:(c + 1) * B] for c in range(K0 // 2)]

        def act(o, ps, scale, bias, bias2, m):
            if zero_bias:
                nc.scalar.activation(o[:], ps[:], PRELU,
                                     scale=scale, alpha=alt[:])
            elif scale == 1.0:
                nc.scalar.activation(o[:], ps[:], PRELU,
                                     bias=bias[:, m:m + 1], alpha=alt[:])
            else:
                # s = lrelu(y+b0) + lrelu(6y+2b0) + lrelu(5y+b0)
                acc = tmp_p.tile([P, B], F32, tag="acc", name=f"acc_{m}")
                first = True
                for sc, bt in ((1.0, bias), (6.0, bias2), (5.0, bias)):
                    l = tmp_p.tile([P, B], F32, tag="l", name=f"l_{m}_{sc}")
                    nc.scalar.activation(l[:], ps[:], PRELU, scale=sc,
                                         bias=bt[:, m:m + 1], alpha=alt[:])
                    if first:
                        acc, l = l, acc
                        first = False
                    else:
                        nc.vector.tensor_add(acc[:], acc[:], l[:])
                nc.vector.tensor_copy(o[:], acc[:])

        def mm_group(ps, wm_m, rhs, cs, start, stop):
            for i, c in enumerate(cs):
                nc.tensor.matmul(
                    ps[:], lhsT=wm_m[:, c * P:(c + 1) * P], rhs=rhs[c],
                    start=(start and i == 0), stop=(stop and i == len(cs) - 1),
                )

        def layer(M, K, wm, rhs, out_pool, scale, bias, bias2, lname):
            outs = []
            for m in range(M):
                ps = ps_p.tile([P, B], F32, tag="ps", name=f"ps_{lname}_{m}")
                mm_group(ps, wm[m], rhs, range(K), True, True)
                o = out_pool.tile([P, B], F16, tag=lname, name=f"{lname}_{m}")
                act(o, ps, scale, bias, bias2, m)
                outs.append(o)
            return outs

        if zero_bias:
            b0t = b0t2 = b1t = b2t = None
        s = layer(M0, K0, w0m, xrhs, s_p, 12.0, b0t, b0t2, "s")

        # ---- layers 2+3: plain loops; L3's c-ascending order means only
        # the c=3 matmuls wait on h3, and the tail is g0-ACT (scalar) in
        # parallel with g1 (DVE), then the classifier.
        srhs = [t[:] for t in s]
        h = layer(M1, K1, w1m, srhs, h_p, 1.0, b1t, None, "h")
        hrhs = [t[:] for t in h]
        g = []
        for m in range(M2):
            ps = g_psp.tile([P, B], F32, tag="gps", name=f"ps_g_{m}")
            mm_group(ps, w2m[m], hrhs, range(K2), True, True)
            o = g_p.tile([P, B], F16, tag="g", name=f"g_{m}")
            if zero_bias and m == 1:
                # g1's lrelu on the DVE (PSUM copy + max) in parallel with
                # g0's scalar-engine Prelu — shortens the serial tail
                c = g_p.tile([P, B], F32, tag="gc", name="g1c")
                nc.vector.tensor_copy(c[:], ps[:])
                nc.vector.scalar_tensor_tensor(
                    o[:], c[:], NEG_SLOPE, c[:],
                    mybir.AluOpType.mult, mybir.AluOpType.max)
            else:
                act(o, ps, 1.0, b2t, None, m)
            g.append(o[:])

        # ---- classifier: out[1, B] = sum_c Wc[c].T @ g[c] (+ bc) ----
        po = cls_ps.tile([1, B], F32)
        for c in range(KC):
            nc.tensor.matmul(
                po[:], lhsT=wc[:, c:c + 1], rhs=g[c],
                start=(c == 0), stop=(c == KC - 1),
            )
        ob = out_p.tile([1, B], F32)
        if zero_bias:
            nc.vector.tensor_copy(ob[:], po[:])
        else:
            nc.vector.tensor_scalar_add(ob[:], po[:], bct[:, 0:1])
        # out rides the (otherwise idle) scalar HWDGE ring so it doesn't
        # queue behind the weight stream's completion descriptors; one
        # packet keeps its completion path minimal
        nc.scalar.dma_start(out_d, ob[:], single_packet=True)

    nc.compile()
    return nc


_CACHE = {}


def _get_nc(zero_bias: bool):
    if zero_bias not in _CACHE:
        _CACHE[zero_bias] = _build(zero_bias)
    return _CACHE[zero_bias]


def _pack_w(w, K, M):
    # [K*128, M*128] -> [128, M*K*128] with col m*K*128 + c*128 + f
    return np.ascontiguousarray(
        w.reshape(K, P, M, P).transpose(1, 2, 0, 3).reshape(P, M * K * P)
    ).astype(np.float16)


def _run(inputs, trace=False, **kw):
    def f32(a):
        return np.ascontiguousarray(np.asarray(a), dtype=np.float32)

    x = f32(inputs["x"])
    W0, b0 = f32(inputs["W0"]), f32(inputs["b0"])
    W1, b1 = f32(inputs["W1"]), f32(inputs["b1"])
    W2, b2 = f32(inputs["W2"]), f32(inputs["b2"])
    Wc, bc = f32(inputs["Wc"]), f32(inputs["bc"])
    zero_bias = not (b0.any() or b1.any() or b2.any() or bc.any())
    nc = _get_nc(zero_bias)

    w0p = _pack_w(W0, K0, M0)
    w1p = _pack_w(W1, K1, M1)
    w2p = _pack_w(W2, K2, M2)
    wcp = np.ascontiguousarray(Wc.reshape(KC, P).T).astype(np.float16)

    in_maps = []
    for i in range(N_CORES):
        xs = x[i * B:(i + 1) * B]  # [B, D0]
        xp = np.ascontiguousarray(
            xs.reshape(B, K0, P).transpose(2, 1, 0).reshape(P, K0 * B)
        ).astype(np.float16)
        m = {"x": xp, "W0": w0p, "W1": w1p, "W2": w2p, "Wc": wcp}
        if not zero_bias:
            m.update({"b0": b0, "b1": b1, "b2": b2, "bc": bc})
        in_maps.append(m)
    res = run_bass_kernel_spmd(nc, in_maps, list(range(N_CORES)),
                               trace=trace, **kw)
    out = np.empty((B_FULL, 1), dtype=np.float32)
    for i in range(N_CORES):
        out[i * B:(i + 1) * B, 0] = res.results[i]["out"][0]
    return out, res


def kernel(**inputs) -> np.ndarray:
    out, _ = _run(inputs)
    return out
